# revision 8
# baseline (speedup 1.0000x reference)
"""HTSAD (event-filtered peephole LSTM) Trainium2 kernel, v3.

Strategy: data-parallel over batch (B=64 -> 8 cores x B_LOC=8), sequential
scan over S=4096 on each core.

vs v1 baseline:
  - All matmuls fp16 (fp32 matmuls execute as 2 HW instructions each and
    disable fast-weight-load; fp16 keeps enough mantissa for the recurrence,
    bf16 does not: measured 4.9e-2 rel err bf16 vs 5.3e-3 fp16).
  - States (h, c) kept fp32; a parallel fp16 copy of h feeds the PE.
  - Host-side prep: weights pre-transposed/converted to fp16 numpy-side and
    shipped in their SBUF layouts; event/vc/vn shipped fp16 in [feat, S, B]
    layout so each chunk loads with one contiguous 2-dim DMA per tensor.
  - bef1/bef3/blin folded into ACT bias columns; gate bias via one K=8
    indicator matmul per PSUM bank (indicator shipped as a host constant).
  - Scan matmul order g -> f,i -> o with per-block stops so ACT work starts
    while the PE finishes the o-gate matmuls.

Per-core layout is fully transposed (feature dims on SBUF partitions, batch
on the free dim):
  gates PSUM: 8 banks of [128, 8 blocks, 8 steps, 8 batch]; block order
  [g0 g1 f0 f1 i0 i1 o0 o1] (half = hidden half of HS=256).
"""

import numpy as np

B_FULL = 64
B_LOC = 8
N_CORES = 8
S = 4096
E, C, NN = 64, 32, 16
EMB, HS, EF, DIM = 128, 256, 128, 64
G4 = 4 * HS
MC = 64              # steps per micro-chunk (gates PSUM capacity)
P = 128

# block order (g,f,i,o) x (half0, half1) -> column offset into the
# [i f g o] gate layout of Wx/Wh/bias
BLK_COL = [2 * HS, 2 * HS + 128, HS, HS + 128, 0, 128, 3 * HS, 3 * HS + 128]
# peephole weight row for [f, i, o] rows of wcbc: f->Wc[1], i->Wc[0], o->Wc[2]
CW_ROWS = [1, 0, 2]


def build_nc(s_total=S, mc=MC):
    import concourse.bass as bass
    import concourse.tile as tile
    import concourse.mybir as mybir
    from concourse import bacc
    from concourse.bass import ds

    fp32 = mybir.dt.float32
    fp16 = mybir.dt.float16
    AF = mybir.ActivationFunctionType
    OP = mybir.AluOpType

    n_chunks = s_total // mc
    NCH_COLS = mc * B_LOC          # 512 cols per chunk (t-major, b-minor)

    nc = bacc.Bacc()

    # inputs already transposed/converted host-side
    event_d = nc.declare_dram_parameter("event", [E, s_total, B_LOC], fp16, isOutput=False)
    vc_d = nc.declare_dram_parameter("vc", [C, s_total, B_LOC], fp16, isOutput=False)
    vn_d = nc.declare_dram_parameter("vn", [NN, s_total, B_LOC], fp16, isOutput=False)
    h0_d = nc.declare_dram_parameter("h0", [P, 2, B_LOC], fp32, isOutput=False)
    h0h_d = nc.declare_dram_parameter("h0h", [P, 2, B_LOC], fp16, isOutput=False)
    c0_d = nc.declare_dram_parameter("c0", [P, 2, B_LOC], fp32, isOutput=False)
    Wx_d = nc.declare_dram_parameter("Wx", [P, G4], fp16, isOutput=False)
    Wh_d = nc.declare_dram_parameter("Wh", [P, 2, G4], fp16, isOutput=False)
    wcbc_d = nc.declare_dram_parameter("wcbc", [P, 3, 2, B_LOC], fp32, isOutput=False)
    bias8_d = nc.declare_dram_parameter("bias8", [8, P], fp16, isOutput=False)
    ind8_d = nc.declare_dram_parameter("ind8", [8, 8, mc // 8, B_LOC], fp16, isOutput=False)
    Ve_d = nc.declare_dram_parameter("Ve", [E, EMB], fp16, isOutput=False)
    Vc2_d = nc.declare_dram_parameter("Vc2", [C, EMB], fp16, isOutput=False)
    Vn_d = nc.declare_dram_parameter("Vn", [NN, EMB], fp16, isOutput=False)
    Wlin_d = nc.declare_dram_parameter("Wlin", [P, 2, DIM], fp32, isOutput=False)
    blin_d = nc.declare_dram_parameter("blin", [DIM, 1], fp32, isOutput=False)
    Wef1_d = nc.declare_dram_parameter("Wef1", [P, EF], fp16, isOutput=False)
    bef1_d = nc.declare_dram_parameter("bef1", [P, 1], fp32, isOutput=False)
    Wef3_d = nc.declare_dram_parameter("Wef3", [P, HS], fp16, isOutput=False)
    bef3_d = nc.declare_dram_parameter("bef3", [P, 2], fp32, isOutput=False)
    out_d = nc.declare_dram_parameter("out", [B_LOC, DIM], fp32, isOutput=True)

    with tile.TileContext(nc) as tc:
        with (
            tc.tile_pool(name="wts", bufs=1) as wts,
            tc.tile_pool(name="state", bufs=1) as stp,
            tc.tile_pool(name="chunk", bufs=2) as chp,
            tc.tile_pool(name="scr", bufs=3) as scr,
            tc.tile_pool(name="psum", bufs=1, space="PSUM") as psp,
        ):
            # ---------------- weights straight into SBUF ----------------
            def ld(nm, shape, dt, src):
                t = wts.tile(shape, dt, name=nm, tag=nm)
                nc.sync.dma_start(t[:], src[:])
                return t

            Wh_sb = ld("Wh_sb", [P, 2, G4], fp16, Wh_d)
            Wx_sb = ld("Wx_sb", [P, G4], fp16, Wx_d)
            Ve_sb = ld("Ve_sb", [E, EMB], fp16, Ve_d)
            Vc2_sb = ld("Vc2_sb", [C, EMB], fp16, Vc2_d)
            Vn_sb = ld("Vn_sb", [NN, EMB], fp16, Vn_d)
            Wef1_sb = ld("Wef1_sb", [P, EF], fp16, Wef1_d)
            Wef3_sb = ld("Wef3_sb", [P, HS], fp16, Wef3_d)
            bias8_sb = ld("bias8_sb", [8, P], fp16, bias8_d)
            ind8_sb = ld("ind8_sb", [8, 8, mc // 8, B_LOC], fp16, ind8_d)
            wcbc = ld("wcbc_sb", [P, 3, 2, B_LOC], fp32, wcbc_d)
            Wlin_sb = ld("Wlin_sb", [P, 2, DIM], fp32, Wlin_d)
            blin_col = ld("blin_col", [DIM, 1], fp32, blin_d)
            bef1_col = ld("bef1_col", [P, 1], fp32, bef1_d)
            bef3_col = ld("bef3_col", [P, 2], fp32, bef3_d)

            # ---------------- state ----------------
            hT = stp.tile([P, 2, B_LOC], fp32)       # [p, half, b]
            hTb = stp.tile([P, 2, B_LOC], fp16)
            # STATE = [c_hat(2,8) | c(2,8) | g(2,8)]
            STATE = stp.tile([P, 3, 2, B_LOC], fp32)
            nc.sync.dma_start(hT[:], h0_d[:])
            nc.sync.dma_start(hTb[:], h0h_d[:])
            nc.sync.dma_start(STATE[:, 1, :, :], c0_d[:])

            # ---------------- main loop over micro-chunks ----------------
            def chunk_body(ci):
                t0 = ci * mc
                # single contiguous DMA per tensor
                evT = chp.tile([E, mc, B_LOC], fp16, tag="evT")
                vcT = chp.tile([C, mc, B_LOC], fp16, tag="vcT")
                vnT = chp.tile([NN, mc, B_LOC], fp16, tag="vnT")
                nc.sync.dma_start(evT[:], event_d[:, ds(t0, mc), :])
                nc.sync.dma_start(vcT[:], vc_d[:, ds(t0, mc), :])
                nc.sync.dma_start(vnT[:], vn_d[:, ds(t0, mc), :])

                banks = []
                for k in range(8):
                    bank_t = psp.tile([P, 8, mc // 8, B_LOC], fp32,
                                      tag=f"bank{k}", name=f"bank{k}")  # [p, blk, t, b]
                    banks.append(bank_t)

                # -------- phase A: s, x, j for the whole chunk --------
                ps_x = banks[0][:].rearrange("p blk t b -> p (blk t b)")  # [128,512]
                ps_h = banks[1][:].rearrange("p blk t b -> p (blk t b)")
                # s = event @ Ve
                nc.tensor.matmul(ps_x, Ve_sb[:], evT[:].rearrange("e t b -> e (t b)"),
                                 start=True, stop=True)
                s_sb = chp.tile([P, NCH_COLS], fp16, tag="s_sb")
                nc.scalar.copy(s_sb[:], ps_x)
                # x = s + 2*vc@Vc + 2*tanh(vn@Vn)
                nc.tensor.matmul(ps_x, Vc2_sb[:], vcT[:].rearrange("c t b -> c (t b)"),
                                 start=False, stop=True, skip_group_check=True)
                nc.tensor.matmul(ps_h, Vn_sb[:], vnT[:].rearrange("n t b -> n (t b)"),
                                 start=True, stop=True)
                tn_sb = chp.tile([P, NCH_COLS], fp32, tag="tn_sb")
                nc.scalar.activation(tn_sb[:], ps_h, AF.Tanh)
                xTb = chp.tile([P, mc, B_LOC], fp16, tag="xTb")
                nc.vector.scalar_tensor_tensor(
                    xTb[:].rearrange("p t b -> p (t b)"), tn_sb[:], 2.0, ps_x,
                    op0=OP.mult, op1=OP.add,
                )
                # u = tanh(s @ Wef1 + bef1)
                nc.tensor.matmul(ps_h, Wef1_sb[:], s_sb[:], start=True, stop=True)
                u_sb = chp.tile([P, NCH_COLS], fp16, tag="u_sb")
                nc.scalar.activation(u_sb[:], ps_h, AF.Tanh, bias=bef1_col[:, 0:1])
                # j = sigmoid(u @ Wef3 + bef3); jmj layout [p, t, (j0 j1 mj0 mj1), b]
                jmj = chp.tile([P, mc, 4, B_LOC], fp32, tag="jmj")
                for hf in range(2):
                    ps_j = banks[2 + hf][:].rearrange("p blk t b -> p (blk t b)")
                    nc.tensor.matmul(ps_j, Wef3_sb[:, hf * P : (hf + 1) * P], u_sb[:],
                                     start=True, stop=True)
                    nc.scalar.activation(jmj[:, :, hf, :], ps_j, AF.Sigmoid,
                                         bias=bef3_col[:, hf:hf + 1])
                # mj = 1 - j
                nc.scalar.activation(jmj[:, :, 2:4, :], jmj[:, :, 0:2, :],
                                     AF.Identity, bias=1.0, scale=-1.0)

                # -------- phase B: bias + x@Wx pre-accumulated into gates ---
                for k in range(8):
                    nc.tensor.matmul(
                        banks[k][:].rearrange("p blk t b -> p (blk t b)"),
                        bias8_sb[:], ind8_sb[:].rearrange("j blk t b -> j (blk t b)"),
                        start=True, stop=False, skip_group_check=True,
                    )
                for blk in range(8):
                    co = BLK_COL[blk]
                    for k in range(8):
                        nc.tensor.matmul(
                            banks[k][:, blk, :, :], Wx_sb[:, co : co + P],
                            xTb[:, (mc // 8) * k : (mc // 8) * k + mc // 8, :],
                            start=False, stop=False, skip_group_check=True,
                        )

                # -------- phase C: the scan --------
                for tl in range(mc):
                    bk = banks[tl // 8]
                    trow = tl % 8
                    jmj_t = jmj[:, tl, :, :]

                    # peephole term cw = c * wcbc for [f0 f1 i0 i1 o0 o1]
                    cw = scr.tile([P, 3, 2, B_LOC], fp32, tag="cw")
                    nc.gpsimd.tensor_mul(
                        cw[:],
                        STATE[:, 1, :, :].unsqueeze(1).to_broadcast([P, 3, 2, B_LOC]),
                        wcbc[:],
                    )
                    # m2 = (1-j) * h   (independent of this step's gates)
                    m2T = scr.tile([P, 2, B_LOC], fp32, tag="m2T")
                    nc.gpsimd.tensor_mul(m2T[:], jmj_t[:, 2:4, :], hT[:])

                    # recurrent matmuls: block order g,g,f,f,i,i,o,o
                    for blk in range(8):
                        co = BLK_COL[blk]
                        for k in range(2):
                            nc.tensor.matmul(
                                bk[:, blk, trow, :], Wh_sb[:, k, co : co + P],
                                hTb[:, k, :],
                                start=False, stop=(k == 1),
                                skip_group_check=True,
                            )

                    # g = tanh(psum) straight off the PE (no peephole on g)
                    nc.scalar.activation(STATE[:, 2, :, :], bk[:, 0:2, trow, :], AF.Tanh)
                    # f,i: pre-activations = gates + cw
                    pre_fi = scr.tile([P, 4, B_LOC], fp32, tag="pre_fi")
                    nc.vector.tensor_add(
                        pre_fi[:], bk[:, 2:6, trow, :],
                        cw[:, 0:2, :, :].rearrange("p r hf b -> p (r hf) b"))
                    fiT = scr.tile([P, 4, B_LOC], fp32, tag="fiT")
                    nc.scalar.activation(fiT[:], pre_fi[:], AF.Sigmoid)
                    # o on gpsimd+ACT in parallel with the c_hat path
                    pre_o = scr.tile([P, 2, B_LOC], fp32, tag="pre_o")
                    nc.vector.tensor_add(pre_o[:], bk[:, 6:8, trow, :], cw[:, 2, :, :])
                    oT = scr.tile([P, 2, B_LOC], fp32, tag="oT")
                    nc.scalar.activation(oT[:], pre_o[:], AF.Sigmoid)
                    # c_hat = f*c + i*g
                    fcig = scr.tile([P, 4, B_LOC], fp32, tag="fcig")
                    nc.vector.tensor_mul(
                        fcig[:], fiT[:],
                        STATE[:, 1:3, :, :].rearrange("p s hf b -> p (s hf) b"))
                    nc.vector.tensor_add(STATE[:, 0, :, :], fcig[:, 0:2, :], fcig[:, 2:4, :])
                    # h_hat pieces
                    thT = scr.tile([P, 2, B_LOC], fp32, tag="thT")
                    nc.scalar.activation(thT[:], STATE[:, 0, :, :], AF.Tanh)
                    joT = scr.tile([P, 2, B_LOC], fp32, tag="joT")
                    nc.gpsimd.tensor_mul(joT[:], jmj_t[:, 0:2, :], oT[:])
                    # c_new = j*c_hat + (1-j)*c
                    jcmj = scr.tile([P, 4, B_LOC], fp32, tag="jcmj")
                    nc.gpsimd.tensor_mul(
                        jcmj[:], jmj_t[:],
                        STATE[:, 0:2, :, :].rearrange("p s hf b -> p (s hf) b"))
                    nc.gpsimd.tensor_add(STATE[:, 1, :, :], jcmj[:, 0:2, :], jcmj[:, 2:4, :])
                    # h_new = jo*th + m2 ; fp16 copy first (feeds the PE)
                    m1T = scr.tile([P, 2, B_LOC], fp32, tag="m1T")
                    nc.vector.tensor_mul(m1T[:], joT[:], thT[:])
                    nc.vector.tensor_add(hTb[:], m1T[:], m2T[:])
                    nc.vector.tensor_add(hT[:], m1T[:], m2T[:])

            if n_chunks > 1:
                with tc.For_i(0, n_chunks, 1,
                              hint_engines=(mybir.EngineType.PE,
                                            mybir.EngineType.Activation,
                                            mybir.EngineType.DVE,
                                            mybir.EngineType.Pool)) as ci:
                    chunk_body(ci)
            else:
                chunk_body(0)

            # ---------------- output projection ----------------
            ps_o = psp.tile([DIM, B_LOC], fp32, tag="bank0")
            for k in range(2):
                nc.tensor.matmul(ps_o[:], Wlin_sb[:, k, :], hT[:, k, :],
                                 start=(k == 0), stop=(k == 1))
            outT = stp.tile([DIM, B_LOC], fp32)
            nc.scalar.activation(outT[:], ps_o[:], AF.Identity, bias=blin_col[:, 0:1])
            nc.sync.dma_start(out_d.rearrange("b d -> d b"), outT[:])

    nc.finalize()
    return nc


_NC_CACHE = {}


def _get_nc(s_total=S, mc=MC):
    key = (s_total, mc)
    if key not in _NC_CACHE:
        _NC_CACHE[key] = build_nc(s_total, mc)
    return _NC_CACHE[key]


def _prep_shared(inputs):
    """Host-side constant prep shared by all cores."""
    f16 = np.float16
    f32 = np.float32
    Wh = np.asarray(inputs["Wh"], f32)           # [256, 1024]
    Wx = np.asarray(inputs["Wx"], f32)
    Wc = np.asarray(inputs["Wc"], f32)
    bias = np.asarray(inputs["bias"], f32)
    sh = {
        "Wh": np.ascontiguousarray(
            Wh.reshape(2, P, G4).transpose(1, 0, 2)).astype(f16),
        "Wx": np.ascontiguousarray(Wx).astype(f16),
        "Ve": np.ascontiguousarray(inputs["Ve"]).astype(f16),
        "Vc2": np.ascontiguousarray(2.0 * np.asarray(inputs["Vc"], f32)).astype(f16),
        "Vn": np.ascontiguousarray(inputs["Vn"]).astype(f16),
        "Wef1": np.ascontiguousarray(inputs["Wef1"]).astype(f16),
        "Wef3": np.ascontiguousarray(inputs["Wef3"]).astype(f16),
        "Wlin": np.ascontiguousarray(
            np.asarray(inputs["Wlin"], f32).reshape(2, P, DIM).transpose(1, 0, 2)),
        "blin": np.ascontiguousarray(np.asarray(inputs["blin"], f32).reshape(DIM, 1)),
        "bef1": np.ascontiguousarray(np.asarray(inputs["bef1"], f32).reshape(P, 1)),
        "bef3": np.ascontiguousarray(
            np.asarray(inputs["bef3"], f32).reshape(2, P).T),
    }
    # bias8[blk, p] = bias[BLK_COL[blk] + p] (block order)
    bias8 = np.stack([bias[co:co + P] for co in BLK_COL])
    sh["bias8"] = np.ascontiguousarray(bias8).astype(f16)
    # ind8[r, blk, t, b] = (r == blk)
    ind8 = np.zeros((8, 8, MC // 8, B_LOC), f32)
    for r in range(8):
        ind8[r, r] = 1.0
    sh["ind8"] = ind8.astype(f16)
    # wcbc[p, r, hf, b] = Wc[CW_ROWS[r], hf*128 + p]
    wcbc = np.zeros((P, 3, 2, B_LOC), f32)
    for r, wr in enumerate(CW_ROWS):
        for hf in range(2):
            wcbc[:, r, hf, :] = Wc[wr, hf * P:(hf + 1) * P][:, None]
    sh["wcbc"] = wcbc
    return sh


def _make_in_maps(inputs, s_total=S):
    f16 = np.float16
    f32 = np.float32
    sh = _prep_shared(inputs)
    ev = np.asarray(inputs["event"], f32)[:, :s_total]   # [B, S, E]
    vc = np.asarray(inputs["vc"], f32)[:, :s_total]
    vn = np.asarray(inputs["vn"], f32)[:, :s_total]
    h0 = np.asarray(inputs["h0"], f32)                   # [B, 256]
    c0 = np.asarray(inputs["c0"], f32)
    per_core = []
    for i in range(N_CORES):
        sl = slice(i * B_LOC, (i + 1) * B_LOC)
        h0T = np.ascontiguousarray(
            h0[sl].reshape(B_LOC, 2, P).transpose(2, 1, 0))   # [P, 2, B]
        c0T = np.ascontiguousarray(
            c0[sl].reshape(B_LOC, 2, P).transpose(2, 1, 0))
        m = dict(sh)
        m["event"] = np.ascontiguousarray(
            ev[sl].transpose(2, 1, 0)).astype(f16)            # [E, S, B]
        m["vc"] = np.ascontiguousarray(vc[sl].transpose(2, 1, 0)).astype(f16)
        m["vn"] = np.ascontiguousarray(vn[sl].transpose(2, 1, 0)).astype(f16)
        m["h0"] = h0T
        m["h0h"] = h0T.astype(f16)
        m["c0"] = c0T
        per_core.append(m)
    return per_core


def run(inputs, s_total=S, mc=MC, trace=False):
    """Returns (out [B_FULL, DIM], exec_time_ns or None)."""
    from concourse.bass_utils import run_bass_kernel_spmd

    nc = _get_nc(s_total, mc)
    in_maps = _make_in_maps(inputs, s_total)
    res = run_bass_kernel_spmd(nc, in_maps, list(range(N_CORES)), trace=trace)
    out = np.concatenate([res.results[i]["out"] for i in range(N_CORES)], axis=0)
    return out, res.exec_time_ns


def kernel(**inputs):
    out, _ = run(inputs)
    return out


# revision 9
# speedup vs baseline: 1.0582x; 1.0582x over previous
"""HTSAD (event-filtered peephole LSTM) Trainium2 kernel, v4.

Strategy: data-parallel over batch (B=64 -> 8 cores x B_LOC=8), sequential
scan over S=4096 on each core.

v4 structure (driven by v3 trace analysis):
  - All matmuls fp16; 16 gate matmuls + 6 diagonal peephole matmuls per step
    pipeline on the PE at ~25ns issue gaps (~0.7us per step).
  - The peephole c*Wc terms are folded into the PSUM accumulation via
    diagonal-stationary matmuls on a fp16 copy of c, so the sigmoids read
    PSUM directly (no pre-add hop).
  - One merged sigmoid over [f,i,o] blocks; tanh(g) runs early off-path.
  - The whole tensor-op chain lives on the DVE so its per-step standalone
    semaphore wait has a single (ACT) condition: the v3 trace showed
    dual-condition waits cost 365ns vs 23-40ns single.
  - States h, c stay fp32 on DVE; fp16 copies (hTb, c16) feed the PE.
  - Weights/constants pre-arranged host-side; event/vc/vn shipped fp16 in
    [feat, S, B] layout so each chunk loads with one contiguous DMA each.

Per-core layout is fully transposed (feature dims on SBUF partitions, batch
on the free dim):
  gates PSUM: 8 banks of [128, 8 blocks, 8 steps, 8 batch]; block order
  [g0 g1 f0 f1 i0 i1 o0 o1] (half = hidden half of HS=256).
"""

import numpy as np

B_FULL = 64
B_LOC = 8
N_CORES = 8
S = 4096
E, C, NN = 64, 32, 16
EMB, HS, EF, DIM = 128, 256, 128, 64
G4 = 4 * HS
MC = 64              # steps per micro-chunk (gates PSUM capacity)
P = 128

# block order (g,f,i,o) x (half0, half1) -> column offset into the
# [i f g o] gate layout of Wx/Wh/bias
BLK_COL = [2 * HS, 2 * HS + 128, HS, HS + 128, 0, 128, 3 * HS, 3 * HS + 128]
# Wc row for the 6 peephole blocks [f0 f1 i0 i1 o0 o1]: f->Wc[1], i->Wc[0],
# o->Wc[2]
DIAG_WC = [1, 1, 0, 0, 2, 2]


def build_nc(s_total=S, mc=MC):
    import concourse.bass as bass
    import concourse.tile as tile
    import concourse.mybir as mybir
    from concourse import bacc
    from concourse.bass import ds

    fp32 = mybir.dt.float32
    fp16 = mybir.dt.float16
    AF = mybir.ActivationFunctionType
    OP = mybir.AluOpType

    n_chunks = s_total // mc
    NCH_COLS = mc * B_LOC          # 512 cols per chunk (t-major, b-minor)

    nc = bacc.Bacc()

    # inputs already transposed/converted host-side
    event_d = nc.declare_dram_parameter("event", [E, s_total, B_LOC], fp16, isOutput=False)
    vc_d = nc.declare_dram_parameter("vc", [C, s_total, B_LOC], fp16, isOutput=False)
    vn_d = nc.declare_dram_parameter("vn", [NN, s_total, B_LOC], fp16, isOutput=False)
    h0_d = nc.declare_dram_parameter("h0", [P, 2, B_LOC], fp32, isOutput=False)
    h0h_d = nc.declare_dram_parameter("h0h", [P, 2, B_LOC], fp16, isOutput=False)
    c0_d = nc.declare_dram_parameter("c0", [P, 2, B_LOC], fp32, isOutput=False)
    c0h_d = nc.declare_dram_parameter("c0h", [P, 2, B_LOC], fp16, isOutput=False)
    Wx_d = nc.declare_dram_parameter("Wx", [P, G4], fp16, isOutput=False)
    Wh_d = nc.declare_dram_parameter("Wh", [P, 2, G4], fp16, isOutput=False)
    dWc_d = nc.declare_dram_parameter("dWc", [P, 6, P], fp16, isOutput=False)
    bias8_d = nc.declare_dram_parameter("bias8", [8, P], fp16, isOutput=False)
    ind8_d = nc.declare_dram_parameter("ind8", [8, 8, mc // 8, B_LOC], fp16, isOutput=False)
    Ve_d = nc.declare_dram_parameter("Ve", [E, EMB], fp16, isOutput=False)
    Vc2_d = nc.declare_dram_parameter("Vc2", [C, EMB], fp16, isOutput=False)
    Vn_d = nc.declare_dram_parameter("Vn", [NN, EMB], fp16, isOutput=False)
    Wlin_d = nc.declare_dram_parameter("Wlin", [P, 2, DIM], fp32, isOutput=False)
    blin_d = nc.declare_dram_parameter("blin", [DIM, 1], fp32, isOutput=False)
    Wef1_d = nc.declare_dram_parameter("Wef1", [P, EF], fp16, isOutput=False)
    bef1_d = nc.declare_dram_parameter("bef1", [P, 1], fp32, isOutput=False)
    Wef3_d = nc.declare_dram_parameter("Wef3", [P, HS], fp16, isOutput=False)
    bef3_d = nc.declare_dram_parameter("bef3", [P, 2], fp32, isOutput=False)
    out_d = nc.declare_dram_parameter("out", [B_LOC, DIM], fp32, isOutput=True)

    with tile.TileContext(nc) as tc:
        with (
            tc.tile_pool(name="wts", bufs=1) as wts,
            tc.tile_pool(name="state", bufs=1) as stp,
            tc.tile_pool(name="chunk", bufs=2) as chp,
            tc.tile_pool(name="scr", bufs=3) as scr,
            tc.tile_pool(name="psum", bufs=1, space="PSUM") as psp,
        ):
            # ---------------- weights straight into SBUF ----------------
            def ld(nm, shape, dt, src):
                t = wts.tile(shape, dt, name=nm, tag=nm)
                nc.sync.dma_start(t[:], src[:])
                return t

            Wh_sb = ld("Wh_sb", [P, 2, G4], fp16, Wh_d)
            Wx_sb = ld("Wx_sb", [P, G4], fp16, Wx_d)
            dWc_sb = ld("dWc_sb", [P, 6, P], fp16, dWc_d)
            Ve_sb = ld("Ve_sb", [E, EMB], fp16, Ve_d)
            Vc2_sb = ld("Vc2_sb", [C, EMB], fp16, Vc2_d)
            Vn_sb = ld("Vn_sb", [NN, EMB], fp16, Vn_d)
            Wef1_sb = ld("Wef1_sb", [P, EF], fp16, Wef1_d)
            Wef3_sb = ld("Wef3_sb", [P, HS], fp16, Wef3_d)
            bias8_sb = ld("bias8_sb", [8, P], fp16, bias8_d)
            ind8_sb = ld("ind8_sb", [8, 8, mc // 8, B_LOC], fp16, ind8_d)
            Wlin_sb = ld("Wlin_sb", [P, 2, DIM], fp32, Wlin_d)
            blin_col = ld("blin_col", [DIM, 1], fp32, blin_d)
            bef1_col = ld("bef1_col", [P, 1], fp32, bef1_d)
            bef3_col = ld("bef3_col", [P, 2], fp32, bef3_d)

            # ---------------- state (all owned by DVE in the scan) -------
            hT = stp.tile([P, 2, B_LOC], fp32)       # [p, half, b]
            hTb = stp.tile([P, 2, B_LOC], fp16)
            c16 = stp.tile([P, 2, B_LOC], fp16)
            # STATE = [c_hat(2,8) | c(2,8) | g(2,8)]
            STATE = stp.tile([P, 3, 2, B_LOC], fp32)
            nc.sync.dma_start(hT[:], h0_d[:])
            nc.sync.dma_start(hTb[:], h0h_d[:])
            nc.sync.dma_start(c16[:], c0h_d[:])
            nc.sync.dma_start(STATE[:, 1, :, :], c0_d[:])

            # ---------------- main loop over micro-chunks ----------------
            def chunk_body(ci):
                t0 = ci * mc
                # single contiguous DMA per tensor
                evT = chp.tile([E, mc, B_LOC], fp16, tag="evT")
                vcT = chp.tile([C, mc, B_LOC], fp16, tag="vcT")
                vnT = chp.tile([NN, mc, B_LOC], fp16, tag="vnT")
                nc.sync.dma_start(evT[:], event_d[:, ds(t0, mc), :])
                nc.sync.dma_start(vcT[:], vc_d[:, ds(t0, mc), :])
                nc.sync.dma_start(vnT[:], vn_d[:, ds(t0, mc), :])

                banks = []
                for k in range(8):
                    bank_t = psp.tile([P, 8, mc // 8, B_LOC], fp32,
                                      tag=f"bank{k}", name=f"bank{k}")  # [p, blk, t, b]
                    banks.append(bank_t)

                # -------- phase A: s, x, j for the whole chunk --------
                ps_x = banks[0][:].rearrange("p blk t b -> p (blk t b)")  # [128,512]
                ps_h = banks[1][:].rearrange("p blk t b -> p (blk t b)")
                # s = event @ Ve
                nc.tensor.matmul(ps_x, Ve_sb[:], evT[:].rearrange("e t b -> e (t b)"),
                                 start=True, stop=True)
                s_sb = chp.tile([P, NCH_COLS], fp16, tag="s_sb")
                nc.scalar.copy(s_sb[:], ps_x)
                # x = s + 2*vc@Vc + 2*tanh(vn@Vn)
                nc.tensor.matmul(ps_x, Vc2_sb[:], vcT[:].rearrange("c t b -> c (t b)"),
                                 start=False, stop=True, skip_group_check=True)
                nc.tensor.matmul(ps_h, Vn_sb[:], vnT[:].rearrange("n t b -> n (t b)"),
                                 start=True, stop=True)
                tn_sb = chp.tile([P, NCH_COLS], fp32, tag="tn_sb")
                nc.scalar.activation(tn_sb[:], ps_h, AF.Tanh)
                xTb = chp.tile([P, mc, B_LOC], fp16, tag="xTb")
                nc.vector.scalar_tensor_tensor(
                    xTb[:].rearrange("p t b -> p (t b)"), tn_sb[:], 2.0, ps_x,
                    op0=OP.mult, op1=OP.add,
                )
                # u = tanh(s @ Wef1 + bef1)
                nc.tensor.matmul(ps_h, Wef1_sb[:], s_sb[:], start=True, stop=True)
                u_sb = chp.tile([P, NCH_COLS], fp16, tag="u_sb")
                nc.scalar.activation(u_sb[:], ps_h, AF.Tanh, bias=bef1_col[:, 0:1])
                # j = sigmoid(u @ Wef3 + bef3); jmj layout [p, t, (j0 j1 mj0 mj1), b]
                jmj = chp.tile([P, mc, 4, B_LOC], fp32, tag="jmj")
                for hf in range(2):
                    ps_j = banks[2 + hf][:].rearrange("p blk t b -> p (blk t b)")
                    nc.tensor.matmul(ps_j, Wef3_sb[:, hf * P : (hf + 1) * P], u_sb[:],
                                     start=True, stop=True)
                    nc.scalar.activation(jmj[:, :, hf, :], ps_j, AF.Sigmoid,
                                         bias=bef3_col[:, hf:hf + 1])
                # mj = 1 - j
                nc.scalar.activation(jmj[:, :, 2:4, :], jmj[:, :, 0:2, :],
                                     AF.Identity, bias=1.0, scale=-1.0)

                # -------- phase B: bias + x@Wx pre-accumulated into gates ---
                for k in range(8):
                    nc.tensor.matmul(
                        banks[k][:].rearrange("p blk t b -> p (blk t b)"),
                        bias8_sb[:], ind8_sb[:].rearrange("j blk t b -> j (blk t b)"),
                        start=True, stop=False, skip_group_check=True,
                    )
                for blk in range(8):
                    co = BLK_COL[blk]
                    for k in range(8):
                        nc.tensor.matmul(
                            banks[k][:, blk, :, :], Wx_sb[:, co : co + P],
                            xTb[:, (mc // 8) * k : (mc // 8) * k + mc // 8, :],
                            start=False, stop=False, skip_group_check=True,
                        )

                # -------- phase C: the scan --------
                for tl in range(mc):
                    bk = banks[tl // 8]
                    trow = tl % 8
                    jmj_t = jmj[:, tl, :, :]

                    # m2 = (1-j)*h on DVE during the matmul batch (no cross-
                    # engine inputs: hT and jmj are DVE-local/chunk-const)
                    m2T = scr.tile([P, 2, B_LOC], fp32, tag="m2T")
                    nc.vector.tensor_mul(m2T[:], jmj_t[:, 2:4, :], hT[:])

                    # gate matmuls g,f,i,o then peephole diagonals
                    for blk in range(8):
                        co = BLK_COL[blk]
                        for k in range(2):
                            nc.tensor.matmul(
                                bk[:, blk, trow, :], Wh_sb[:, k, co : co + P],
                                hTb[:, k, :],
                                start=False, stop=(blk < 2 and k == 1),
                                skip_group_check=True,
                            )
                    for dr in range(6):
                        nc.tensor.matmul(
                            bk[:, 2 + dr, trow, :], dWc_sb[:, dr, :],
                            c16[:, dr % 2, :],
                            start=False, stop=True, skip_group_check=True,
                        )

                    # ACT: g early, merged sigmoid(f,i,o), later tanh(c_hat)
                    nc.scalar.activation(STATE[:, 2, :, :], bk[:, 0:2, trow, :], AF.Tanh)
                    fio = scr.tile([P, 6, B_LOC], fp32, tag="fio")
                    nc.scalar.activation(fio[:], bk[:, 2:8, trow, :], AF.Sigmoid)

                    # DVE chain
                    fcig = scr.tile([P, 4, B_LOC], fp32, tag="fcig")
                    nc.vector.tensor_mul(
                        fcig[:], fio[:, 0:4, :],
                        STATE[:, 1:3, :, :].rearrange("p s hf b -> p (s hf) b"))
                    nc.vector.tensor_add(STATE[:, 0, :, :], fcig[:, 0:2, :], fcig[:, 2:4, :])
                    joT = scr.tile([P, 2, B_LOC], fp32, tag="joT")
                    nc.vector.tensor_mul(joT[:], jmj_t[:, 0:2, :], fio[:, 4:6, :])
                    thT = scr.tile([P, 2, B_LOC], fp32, tag="thT")
                    nc.scalar.activation(thT[:], STATE[:, 0, :, :], AF.Tanh)
                    m1T = scr.tile([P, 2, B_LOC], fp32, tag="m1T")
                    nc.vector.tensor_mul(m1T[:], joT[:], thT[:])
                    nc.vector.tensor_add(hTb[:], m1T[:], m2T[:])
                    # post-critical tail, still DVE: c path + fp32 h
                    jcmj = scr.tile([P, 4, B_LOC], fp32, tag="jcmj")
                    nc.vector.tensor_mul(
                        jcmj[:], jmj_t[:],
                        STATE[:, 0:2, :, :].rearrange("p s hf b -> p (s hf) b"))
                    nc.vector.tensor_add(STATE[:, 1, :, :], jcmj[:, 0:2, :], jcmj[:, 2:4, :])
                    nc.vector.tensor_copy(c16[:], STATE[:, 1, :, :])
                    nc.vector.tensor_add(hT[:], m1T[:], m2T[:])

            if n_chunks > 1:
                with tc.For_i(0, n_chunks, 1,
                              hint_engines=(mybir.EngineType.PE,
                                            mybir.EngineType.Activation,
                                            mybir.EngineType.DVE,
                                            mybir.EngineType.Pool)) as ci:
                    chunk_body(ci)
            else:
                chunk_body(0)

            # ---------------- output projection ----------------
            ps_o = psp.tile([DIM, B_LOC], fp32, tag="bank0")
            for k in range(2):
                nc.tensor.matmul(ps_o[:], Wlin_sb[:, k, :], hT[:, k, :],
                                 start=(k == 0), stop=(k == 1))
            outT = stp.tile([DIM, B_LOC], fp32)
            nc.scalar.activation(outT[:], ps_o[:], AF.Identity, bias=blin_col[:, 0:1])
            nc.sync.dma_start(out_d.rearrange("b d -> d b"), outT[:])

    nc.finalize()
    return nc


_NC_CACHE = {}


def _get_nc(s_total=S, mc=MC):
    key = (s_total, mc)
    if key not in _NC_CACHE:
        _NC_CACHE[key] = build_nc(s_total, mc)
    return _NC_CACHE[key]


def _prep_shared(inputs):
    """Host-side constant prep shared by all cores."""
    f16 = np.float16
    f32 = np.float32
    Wh = np.asarray(inputs["Wh"], f32)           # [256, 1024]
    Wx = np.asarray(inputs["Wx"], f32)
    Wc = np.asarray(inputs["Wc"], f32)
    bias = np.asarray(inputs["bias"], f32)
    sh = {
        "Wh": np.ascontiguousarray(
            Wh.reshape(2, P, G4).transpose(1, 0, 2)).astype(f16),
        "Wx": np.ascontiguousarray(Wx).astype(f16),
        "Ve": np.ascontiguousarray(inputs["Ve"]).astype(f16),
        "Vc2": np.ascontiguousarray(2.0 * np.asarray(inputs["Vc"], f32)).astype(f16),
        "Vn": np.ascontiguousarray(inputs["Vn"]).astype(f16),
        "Wef1": np.ascontiguousarray(inputs["Wef1"]).astype(f16),
        "Wef3": np.ascontiguousarray(inputs["Wef3"]).astype(f16),
        "Wlin": np.ascontiguousarray(
            np.asarray(inputs["Wlin"], f32).reshape(2, P, DIM).transpose(1, 0, 2)),
        "blin": np.ascontiguousarray(np.asarray(inputs["blin"], f32).reshape(DIM, 1)),
        "bef1": np.ascontiguousarray(np.asarray(inputs["bef1"], f32).reshape(P, 1)),
        "bef3": np.ascontiguousarray(
            np.asarray(inputs["bef3"], f32).reshape(2, P).T),
    }
    # bias8[blk, p] = bias[BLK_COL[blk] + p] (block order)
    bias8 = np.stack([bias[co:co + P] for co in BLK_COL])
    sh["bias8"] = np.ascontiguousarray(bias8).astype(f16)
    # ind8[r, blk, t, b] = (r == blk)
    ind8 = np.zeros((8, 8, MC // 8, B_LOC), f32)
    for r in range(8):
        ind8[r, r] = 1.0
    sh["ind8"] = ind8.astype(f16)
    # dWc[p, dr, q] = (p==q) * Wc[DIAG_WC[dr], (dr%2)*128 + p]
    dWc = np.zeros((P, 6, P), f32)
    for dr in range(6):
        np.fill_diagonal(dWc[:, dr, :], Wc[DIAG_WC[dr], (dr % 2) * P:(dr % 2 + 1) * P])
    sh["dWc"] = dWc.astype(f16)
    return sh


def _make_in_maps(inputs, s_total=S):
    f16 = np.float16
    f32 = np.float32
    sh = _prep_shared(inputs)
    ev = np.asarray(inputs["event"], f32)[:, :s_total]   # [B, S, E]
    vc = np.asarray(inputs["vc"], f32)[:, :s_total]
    vn = np.asarray(inputs["vn"], f32)[:, :s_total]
    h0 = np.asarray(inputs["h0"], f32)                   # [B, 256]
    c0 = np.asarray(inputs["c0"], f32)
    per_core = []
    for i in range(N_CORES):
        sl = slice(i * B_LOC, (i + 1) * B_LOC)
        h0T = np.ascontiguousarray(
            h0[sl].reshape(B_LOC, 2, P).transpose(2, 1, 0))   # [P, 2, B]
        c0T = np.ascontiguousarray(
            c0[sl].reshape(B_LOC, 2, P).transpose(2, 1, 0))
        m = dict(sh)
        m["event"] = np.ascontiguousarray(
            ev[sl].transpose(2, 1, 0)).astype(f16)            # [E, S, B]
        m["vc"] = np.ascontiguousarray(vc[sl].transpose(2, 1, 0)).astype(f16)
        m["vn"] = np.ascontiguousarray(vn[sl].transpose(2, 1, 0)).astype(f16)
        m["h0"] = h0T
        m["h0h"] = h0T.astype(f16)
        m["c0"] = c0T
        m["c0h"] = c0T.astype(f16)
        per_core.append(m)
    return per_core


def run(inputs, s_total=S, mc=MC, trace=False):
    """Returns (out [B_FULL, DIM], exec_time_ns or None)."""
    from concourse.bass_utils import run_bass_kernel_spmd

    nc = _get_nc(s_total, mc)
    in_maps = _make_in_maps(inputs, s_total)
    res = run_bass_kernel_spmd(nc, in_maps, list(range(N_CORES)), trace=trace)
    out = np.concatenate([res.results[i]["out"] for i in range(N_CORES)], axis=0)
    return out, res.exec_time_ns


def kernel(**inputs):
    out, _ = run(inputs)
    return out


# revision 11
# speedup vs baseline: 1.1977x; 1.1318x over previous
"""HTSAD (event-filtered peephole LSTM) Trainium2 kernel, v5b.

Strategy: data-parallel over batch (B=64 -> 8 cores x B_LOC=8), sequential
scan over S=4096 on each core.

Key structure (driven by trace analysis of earlier versions):
  - All matmuls fp16. Per step: 16 gate matmuls + 1 identity-stationary
    matmul that adds the precomputed fp16 peephole term cw6 = c*Wc into
    PSUM, so one merged sigmoid reads PSUM directly.
  - g-block weights pre-scaled x2 host-side: tanh(pre_g) = 2*sig(2 pre_g)-1,
    recovered with a single tensor_scalar, so the step needs only two ACT
    ops (sigmoid over all 8 blocks, tanh(c_hat)).
  - The whole tensor-op chain lives on the DVE (single-condition semaphore
    waits; dual-condition standalone waits measured 365ns vs 23-40ns).
  - Phase A/B (projections, j-gate, bias+x@Wx pre-accumulation) for chunk
    c+1 is interleaved into scan(c)'s idle engine slots; the For_i body
    holds two chunks (ping-pong tile sets) so the interleave is static.
  - Copy/Identity ops kept off ACT so the sigmoid/tanh tables stay
    resident (table reloads cost 1.3us each at every chunk boundary).
  - States h, c stay fp32; fp16 copies (hTb, cw6) feed the PE.
  - Weights/constants pre-arranged host-side; event/vc/vn shipped fp16 in
    [feat, S, B] layout so each chunk loads with one contiguous DMA each.

Per-core layout is fully transposed (feature dims on SBUF partitions, batch
on the free dim):
  gates PSUM: 8 banks of [128, 8 blocks, 8 steps, 8 batch]; block order
  [g0 g1 f0 f1 i0 i1 o0 o1] (half = hidden half of HS=256).
"""

import numpy as np

B_FULL = 64
B_LOC = 8
N_CORES = 8
S = 4096
E, C, NN = 64, 32, 16
EMB, HS, EF, DIM = 128, 256, 128, 64
G4 = 4 * HS
MC = 64              # steps per micro-chunk (gates PSUM capacity)
P = 128

# block order (g,f,i,o) x (half0, half1) -> column offset into the
# [i f g o] gate layout of Wx/Wh/bias
BLK_COL = [2 * HS, 2 * HS + 128, HS, HS + 128, 0, 128, 3 * HS, 3 * HS + 128]


def build_nc(s_total=S, mc=MC):
    import concourse.bass as bass
    import concourse.tile as tile
    import concourse.mybir as mybir
    from concourse import bacc
    from concourse.bass import ds

    fp32 = mybir.dt.float32
    fp16 = mybir.dt.float16
    AF = mybir.ActivationFunctionType
    OP = mybir.AluOpType

    n_chunks = s_total // mc
    assert n_chunks % 2 == 0
    NCH_COLS = mc * B_LOC          # 512 cols per chunk (t-major, b-minor)

    nc = bacc.Bacc()

    # inputs already transposed/converted host-side
    event_d = nc.declare_dram_parameter("event", [E, s_total, B_LOC], fp16, isOutput=False)
    vc_d = nc.declare_dram_parameter("vc", [C, s_total, B_LOC], fp16, isOutput=False)
    vn_d = nc.declare_dram_parameter("vn", [NN, s_total, B_LOC], fp16, isOutput=False)
    h0_d = nc.declare_dram_parameter("h0", [P, 2, B_LOC], fp32, isOutput=False)
    h0h_d = nc.declare_dram_parameter("h0h", [P, 2, B_LOC], fp16, isOutput=False)
    c0_d = nc.declare_dram_parameter("c0", [P, 2, B_LOC], fp32, isOutput=False)
    cw0_d = nc.declare_dram_parameter("cw0", [P, 3, 2, B_LOC], fp16, isOutput=False)
    Wx_d = nc.declare_dram_parameter("Wx", [P, G4], fp16, isOutput=False)
    Wh_d = nc.declare_dram_parameter("Wh", [P, 2, G4], fp16, isOutput=False)
    ident_d = nc.declare_dram_parameter("ident", [P, P], fp16, isOutput=False)
    wcbc_d = nc.declare_dram_parameter("wcbc", [P, 3, 2, B_LOC], fp16, isOutput=False)
    bias8_d = nc.declare_dram_parameter("bias8", [8, P], fp16, isOutput=False)
    ind8_d = nc.declare_dram_parameter("ind8", [8, 8, mc // 8, B_LOC], fp16, isOutput=False)
    Ve_d = nc.declare_dram_parameter("Ve", [E, EMB], fp16, isOutput=False)
    Vc2_d = nc.declare_dram_parameter("Vc2", [C, EMB], fp16, isOutput=False)
    Vn_d = nc.declare_dram_parameter("Vn", [NN, EMB], fp16, isOutput=False)
    Wlin_d = nc.declare_dram_parameter("Wlin", [P, 2, DIM], fp32, isOutput=False)
    blin_d = nc.declare_dram_parameter("blin", [DIM, 1], fp32, isOutput=False)
    Wef1_d = nc.declare_dram_parameter("Wef1", [P, EF], fp16, isOutput=False)
    bef1_d = nc.declare_dram_parameter("bef1", [P, 1], fp32, isOutput=False)
    Wef3_d = nc.declare_dram_parameter("Wef3", [P, HS], fp16, isOutput=False)
    bef3_d = nc.declare_dram_parameter("bef3", [P, 2], fp32, isOutput=False)
    out_d = nc.declare_dram_parameter("out", [B_LOC, DIM], fp32, isOutput=True)

    with tile.TileContext(nc) as tc:
        with (
            tc.tile_pool(name="wts", bufs=1) as wts,
            tc.tile_pool(name="state", bufs=1) as stp,
            tc.tile_pool(name="chunk", bufs=1) as chp,
            tc.tile_pool(name="scr", bufs=3) as scr,
            tc.tile_pool(name="psum", bufs=1, space="PSUM") as psp,
        ):
            # ---------------- weights straight into SBUF ----------------
            def ldw(nm, shape, dt, src):
                t = wts.tile(shape, dt, name=nm, tag=nm)
                nc.sync.dma_start(t[:], src[:])
                return t

            Wh_sb = ldw("Wh_sb", [P, 2, G4], fp16, Wh_d)
            Wx_sb = ldw("Wx_sb", [P, G4], fp16, Wx_d)
            ident_sb = ldw("ident_sb", [P, P], fp16, ident_d)
            wcbc_sb = ldw("wcbc_sb", [P, 3, 2, B_LOC], fp16, wcbc_d)
            Ve_sb = ldw("Ve_sb", [E, EMB], fp16, Ve_d)
            Vc2_sb = ldw("Vc2_sb", [C, EMB], fp16, Vc2_d)
            Vn_sb = ldw("Vn_sb", [NN, EMB], fp16, Vn_d)
            Wef1_sb = ldw("Wef1_sb", [P, EF], fp16, Wef1_d)
            Wef3_sb = ldw("Wef3_sb", [P, HS], fp16, Wef3_d)
            bias8_sb = ldw("bias8_sb", [8, P], fp16, bias8_d)
            ind8_sb = ldw("ind8_sb", [8, 8, mc // 8, B_LOC], fp16, ind8_d)
            Wlin_sb = ldw("Wlin_sb", [P, 2, DIM], fp32, Wlin_d)
            blin_col = ldw("blin_col", [DIM, 1], fp32, blin_d)
            bef1_col = ldw("bef1_col", [P, 1], fp32, bef1_d)
            bef3_col = ldw("bef3_col", [P, 2], fp32, bef3_d)

            # ---------------- state (all owned by DVE in the scan) -------
            hT = stp.tile([P, 2, B_LOC], fp32)       # [p, half, b]
            hTb = stp.tile([P, 2, B_LOC], fp16)
            cw6 = stp.tile([P, 3, 2, B_LOC], fp16)
            # STATE = [c_hat(2,8) | c(2,8) | G(2,8)]
            STATE = stp.tile([P, 3, 2, B_LOC], fp32)
            nc.sync.dma_start(hT[:], h0_d[:])
            nc.sync.dma_start(hTb[:], h0h_d[:])
            nc.sync.dma_start(cw6[:], cw0_d[:])
            nc.sync.dma_start(STATE[:, 1, :, :], c0_d[:])

            # ---------------- ping-pong chunk tile sets ------------------
            def mkset(p):
                return dict(
                    evT=chp.tile([E, mc, B_LOC], fp16, name=f"evT{p}", tag=f"evT{p}"),
                    vcT=chp.tile([C, mc, B_LOC], fp16, name=f"vcT{p}", tag=f"vcT{p}"),
                    vnT=chp.tile([NN, mc, B_LOC], fp16, name=f"vnT{p}", tag=f"vnT{p}"),
                    s_sb=chp.tile([P, NCH_COLS], fp16, name=f"s_sb{p}", tag=f"s_sb{p}"),
                    tn_sb=chp.tile([P, NCH_COLS], fp32, name=f"tn_sb{p}", tag=f"tn_sb{p}"),
                    u_sb=chp.tile([P, NCH_COLS], fp16, name=f"u_sb{p}", tag=f"u_sb{p}"),
                    xTb=chp.tile([P, mc, B_LOC], fp16, name=f"xTb{p}", tag=f"xTb{p}"),
                    jmj=chp.tile([P, mc, 4, B_LOC], fp32, name=f"jmj{p}", tag=f"jmj{p}"),
                )

            SETS = [mkset(0), mkset(1)]
            banks = []
            for k in range(8):
                banks.append(psp.tile([P, 8, mc // 8, B_LOC], fp32,
                                      tag=f"bank{k}", name=f"bank{k}"))

            # ---------------- phase emitters (for chunk at t0e, set st) ---
            def ph_dma(st, t0e):
                nc.sync.dma_start(st["evT"][:], event_d[:, t0e, :])
                nc.sync.dma_start(st["vcT"][:], vc_d[:, t0e, :])
                nc.sync.dma_start(st["vnT"][:], vn_d[:, t0e, :])

            def ph_a1(st):
                ps_x = banks[0][:].rearrange("p blk t b -> p (blk t b)")
                ps_h = banks[1][:].rearrange("p blk t b -> p (blk t b)")
                nc.tensor.matmul(ps_x, Ve_sb[:],
                                 st["evT"][:].rearrange("e t b -> e (t b)"),
                                 start=True, stop=True, skip_group_check=True)
                nc.vector.tensor_copy(st["s_sb"][:], ps_x)
                nc.tensor.matmul(ps_x, Vc2_sb[:],
                                 st["vcT"][:].rearrange("c t b -> c (t b)"),
                                 start=False, stop=True, skip_group_check=True)
                nc.tensor.matmul(ps_h, Vn_sb[:],
                                 st["vnT"][:].rearrange("n t b -> n (t b)"),
                                 start=True, stop=True, skip_group_check=True)
                nc.scalar.activation(st["tn_sb"][:], ps_h, AF.Tanh)
                nc.vector.scalar_tensor_tensor(
                    st["xTb"][:].rearrange("p t b -> p (t b)"),
                    st["tn_sb"][:], 2.0, ps_x, op0=OP.mult, op1=OP.add)

            def ph_a2(st):
                ps_h = banks[1][:].rearrange("p blk t b -> p (blk t b)")
                nc.tensor.matmul(ps_h, Wef1_sb[:], st["s_sb"][:],
                                 start=True, stop=True, skip_group_check=True)
                nc.scalar.activation(st["u_sb"][:], ps_h, AF.Tanh,
                                     bias=bef1_col[:, 0:1])

            def ph_a3(st, hf):
                ps_j = banks[2 + hf][:].rearrange("p blk t b -> p (blk t b)")
                nc.tensor.matmul(ps_j, Wef3_sb[:, hf * P:(hf + 1) * P], st["u_sb"][:],
                                 start=True, stop=True, skip_group_check=True)
                nc.scalar.activation(st["jmj"][:, :, hf, :], ps_j, AF.Sigmoid,
                                     bias=bef3_col[:, hf:hf + 1])

            def ph_mj(st):
                nc.vector.tensor_scalar(st["jmj"][:, :, 2:4, :],
                                        st["jmj"][:, :, 0:2, :],
                                        -1.0, 1.0, op0=OP.mult, op1=OP.add)

            def ph_b(st, k):
                nc.tensor.matmul(
                    banks[k][:].rearrange("p blk t b -> p (blk t b)"),
                    bias8_sb[:], ind8_sb[:].rearrange("j blk t b -> j (blk t b)"),
                    start=True, stop=False, skip_group_check=True)
                for blk in range(8):
                    co = BLK_COL[blk]
                    nc.tensor.matmul(
                        banks[k][:, blk, :, :], Wx_sb[:, co:co + P],
                        st["xTb"][:, (mc // 8) * k:(mc // 8) * k + mc // 8, :],
                        start=False, stop=False, skip_group_check=True)

            def phase_all(st, t0e):
                ph_dma(st, t0e)
                ph_a1(st)
                ph_a2(st)
                ph_a3(st, 0)
                ph_a3(st, 1)
                ph_mj(st)
                for k in range(8):
                    ph_b(st, k)

            # ---------------- one scan step --------------------------------
            def scan_step(st, tl):
                bk = banks[tl // 8]
                trow = tl % 8
                jmj_t = st["jmj"][:, tl, :, :]

                m2T = scr.tile([P, 2, B_LOC], fp32, tag="m2T")
                nc.vector.tensor_mul(m2T[:], jmj_t[:, 2:4, :], hT[:])

                for blk in range(8):
                    co = BLK_COL[blk]
                    for k in range(2):
                        nc.tensor.matmul(
                            bk[:, blk, trow, :], Wh_sb[:, k, co:co + P],
                            hTb[:, k, :],
                            start=False, stop=(blk < 2 and k == 1),
                            skip_group_check=True)
                nc.tensor.matmul(
                    bk[:, 2:8, trow, :], ident_sb[:],
                    cw6[:].rearrange("p r hf b -> p (r hf b)"),
                    start=False, stop=True, skip_group_check=True)

                S8 = scr.tile([P, 8, B_LOC], fp32, tag="S8")
                nc.scalar.activation(S8[:], bk[:, :, trow, :], AF.Sigmoid)

                nc.vector.tensor_scalar(STATE[:, 2, :, :], S8[:, 0:2, :],
                                        2.0, -1.0, op0=OP.mult, op1=OP.add)
                fcig = scr.tile([P, 4, B_LOC], fp32, tag="fcig")
                nc.vector.tensor_mul(
                    fcig[:], S8[:, 2:6, :],
                    STATE[:, 1:3, :, :].rearrange("p s hf b -> p (s hf) b"))
                nc.vector.tensor_add(STATE[:, 0, :, :], fcig[:, 0:2, :], fcig[:, 2:4, :])
                joT = scr.tile([P, 2, B_LOC], fp32, tag="joT")
                nc.vector.tensor_mul(joT[:], jmj_t[:, 0:2, :], S8[:, 6:8, :])
                jcmj = scr.tile([P, 4, B_LOC], fp32, tag="jcmj")
                nc.vector.tensor_mul(
                    jcmj[:], jmj_t[:],
                    STATE[:, 0:2, :, :].rearrange("p s hf b -> p (s hf) b"))
                thT = scr.tile([P, 2, B_LOC], fp32, tag="thT")
                nc.scalar.activation(thT[:], STATE[:, 0, :, :], AF.Tanh)
                m1T = scr.tile([P, 2, B_LOC], fp32, tag="m1T")
                nc.vector.tensor_mul(m1T[:], joT[:], thT[:])
                nc.vector.tensor_add(hTb[:], m1T[:], m2T[:])
                # post-critical tail, still DVE
                nc.vector.tensor_add(STATE[:, 1, :, :], jcmj[:, 0:2, :], jcmj[:, 2:4, :])
                nc.vector.tensor_mul(
                    cw6[:],
                    STATE[:, 1, :, :].unsqueeze(1).to_broadcast([P, 3, 2, B_LOC]),
                    wcbc_sb[:])
                nc.vector.tensor_add(hT[:], m1T[:], m2T[:])

            # scan chunk reading set `sp`, interleaving phase for the next
            # chunk into set 1-sp (next_t0e None => no interleave)
            def scan_chunk(sp, next_t0e):
                st = SETS[sp]
                nst = SETS[1 - sp]
                if next_t0e is not None:
                    ph_dma(nst, next_t0e)
                for tl in range(mc):
                    scan_step(st, tl)
                    if next_t0e is None:
                        continue
                    if tl == 15:
                        ph_a1(nst)
                    elif tl == 23:
                        ph_a2(nst)
                    elif tl == 31:
                        ph_a3(nst, 0)
                    elif tl == 35:
                        ph_a3(nst, 1)
                    elif tl == 39:
                        ph_mj(nst)
                        ph_b(nst, 0)
                        ph_b(nst, 1)
                    elif tl == 47:
                        ph_b(nst, 2)
                        ph_b(nst, 3)
                        ph_b(nst, 4)
                    elif tl == 55:
                        ph_b(nst, 5)
                        ph_b(nst, 6)
                    elif tl == 63:
                        ph_b(nst, 7)

            # ---------------- schedule -------------------------------------
            # prologue: phase(0)
            phase_all(SETS[0], ds(0, mc))
            # loop covers scans 0..n-3 (2 per iteration), phases 1..n-2
            n_iter = (n_chunks - 2) // 2
            with tc.For_i(0, n_iter, 1,
                          hint_engines=(mybir.EngineType.PE,
                                        mybir.EngineType.Activation,
                                        mybir.EngineType.DVE,
                                        mybir.EngineType.Pool)) as m:
                scan_chunk(0, ds(m * (2 * mc) + mc, mc))
                scan_chunk(1, ds(m * (2 * mc) + 2 * mc, mc))
            # epilogue: scan(n-2) + phase(n-1), scan(n-1)
            scan_chunk(0, ds((n_chunks - 1) * mc, mc))
            scan_chunk(1, None)

            # ---------------- output projection ----------------
            ps_o = psp.tile([DIM, B_LOC], fp32, tag="bank0")
            for k in range(2):
                nc.tensor.matmul(ps_o[:], Wlin_sb[:, k, :], hT[:, k, :],
                                 start=(k == 0), stop=(k == 1))
            outT = stp.tile([DIM, B_LOC], fp32)
            nc.scalar.activation(outT[:], ps_o[:], AF.Identity, bias=blin_col[:, 0:1])
            nc.sync.dma_start(out_d.rearrange("b d -> d b"), outT[:])

    nc.finalize()
    return nc


_NC_CACHE = {}


def _get_nc(s_total=S, mc=MC):
    key = (s_total, mc)
    if key not in _NC_CACHE:
        _NC_CACHE[key] = build_nc(s_total, mc)
    return _NC_CACHE[key]


def _prep_shared(inputs):
    """Host-side constant prep shared by all cores."""
    f16 = np.float16
    f32 = np.float32
    Wh = np.asarray(inputs["Wh"], f32).copy()    # [256, 1024]
    Wx = np.asarray(inputs["Wx"], f32).copy()
    Wc = np.asarray(inputs["Wc"], f32)
    bias = np.asarray(inputs["bias"], f32).copy()
    # g-block pre-activations scaled x2: tanh(x) = 2*sigmoid(2x) - 1
    Wh[:, 2 * HS:3 * HS] *= 2.0
    Wx[:, 2 * HS:3 * HS] *= 2.0
    bias[2 * HS:3 * HS] *= 2.0
    sh = {
        "Wh": np.ascontiguousarray(
            Wh.reshape(2, P, G4).transpose(1, 0, 2)).astype(f16),
        "Wx": np.ascontiguousarray(Wx).astype(f16),
        "Ve": np.ascontiguousarray(inputs["Ve"]).astype(f16),
        "Vc2": np.ascontiguousarray(2.0 * np.asarray(inputs["Vc"], f32)).astype(f16),
        "Vn": np.ascontiguousarray(inputs["Vn"]).astype(f16),
        "Wef1": np.ascontiguousarray(inputs["Wef1"]).astype(f16),
        "Wef3": np.ascontiguousarray(inputs["Wef3"]).astype(f16),
        "Wlin": np.ascontiguousarray(
            np.asarray(inputs["Wlin"], f32).reshape(2, P, DIM).transpose(1, 0, 2)),
        "blin": np.ascontiguousarray(np.asarray(inputs["blin"], f32).reshape(DIM, 1)),
        "bef1": np.ascontiguousarray(np.asarray(inputs["bef1"], f32).reshape(P, 1)),
        "bef3": np.ascontiguousarray(
            np.asarray(inputs["bef3"], f32).reshape(2, P).T),
    }
    # bias8[blk, p] = bias[BLK_COL[blk] + p] (block order)
    bias8 = np.stack([bias[co:co + P] for co in BLK_COL])
    sh["bias8"] = np.ascontiguousarray(bias8).astype(f16)
    # ind8[r, blk, t, b] = (r == blk)
    ind8 = np.zeros((8, 8, MC // 8, B_LOC), f32)
    for r in range(8):
        ind8[r, r] = 1.0
    sh["ind8"] = ind8.astype(f16)
    sh["ident"] = np.eye(P, dtype=f16)
    # wcbc[p, r, hf, b] = Wc[row_r, hf*128+p] for rows [f,i,o]
    wcbc = np.zeros((P, 3, 2, B_LOC), f32)
    for r, wr in enumerate([1, 0, 2]):
        for hf in range(2):
            wcbc[:, r, hf, :] = Wc[wr, hf * P:(hf + 1) * P][:, None]
    sh["wcbc"] = wcbc.astype(f16)
    sh["_wcbc_f32"] = wcbc
    return sh


def _make_in_maps(inputs, s_total=S):
    f16 = np.float16
    f32 = np.float32
    sh = _prep_shared(inputs)
    ev = np.asarray(inputs["event"], f32)[:, :s_total]   # [B, S, E]
    vc = np.asarray(inputs["vc"], f32)[:, :s_total]
    vn = np.asarray(inputs["vn"], f32)[:, :s_total]
    h0 = np.asarray(inputs["h0"], f32)                   # [B, 256]
    c0 = np.asarray(inputs["c0"], f32)
    per_core = []
    for i in range(N_CORES):
        sl = slice(i * B_LOC, (i + 1) * B_LOC)
        h0T = np.ascontiguousarray(
            h0[sl].reshape(B_LOC, 2, P).transpose(2, 1, 0))   # [P, 2, B]
        c0T = np.ascontiguousarray(
            c0[sl].reshape(B_LOC, 2, P).transpose(2, 1, 0))
        m = dict(sh)
        m["event"] = np.ascontiguousarray(
            ev[sl].transpose(2, 1, 0)).astype(f16)            # [E, S, B]
        m["vc"] = np.ascontiguousarray(vc[sl].transpose(2, 1, 0)).astype(f16)
        m["vn"] = np.ascontiguousarray(vn[sl].transpose(2, 1, 0)).astype(f16)
        m["h0"] = h0T
        m["h0h"] = h0T.astype(f16)
        m["c0"] = c0T
        m["cw0"] = (c0T[:, None, :, :] * sh["_wcbc_f32"]).astype(f16)
        per_core.append(m)
    return per_core


def run(inputs, s_total=S, mc=MC, trace=False):
    """Returns (out [B_FULL, DIM], exec_time_ns or None)."""
    from concourse.bass_utils import run_bass_kernel_spmd

    nc = _get_nc(s_total, mc)
    in_maps = _make_in_maps(inputs, s_total)
    res = run_bass_kernel_spmd(nc, in_maps, list(range(N_CORES)), trace=trace)
    out = np.concatenate([res.results[i]["out"] for i in range(N_CORES)], axis=0)
    return out, res.exec_time_ns


def kernel(**inputs):
    out, _ = run(inputs)
    return out


# revision 12
# speedup vs baseline: 1.2003x; 1.0022x over previous
"""HTSAD (event-filtered peephole LSTM) Trainium2 kernel, v5b.

Strategy: data-parallel over batch (B=64 -> 8 cores x B_LOC=8), sequential
scan over S=4096 on each core.

Key structure (driven by trace analysis of earlier versions):
  - All matmuls fp16. Per step: 16 gate matmuls + 1 identity-stationary
    matmul that adds the precomputed fp16 peephole term cw6 = c*Wc into
    PSUM, so one merged sigmoid reads PSUM directly.
  - g-block weights pre-scaled x2 host-side: tanh(pre_g) = 2*sig(2 pre_g)-1,
    recovered with a single tensor_scalar, so the step needs only two ACT
    ops (sigmoid over all 8 blocks, tanh(c_hat)).
  - The whole tensor-op chain lives on the DVE (single-condition semaphore
    waits; dual-condition standalone waits measured 365ns vs 23-40ns).
  - Phase A/B (projections, j-gate, bias+x@Wx pre-accumulation) for chunk
    c+1 is interleaved into scan(c)'s idle engine slots; the For_i body
    holds two chunks (ping-pong tile sets) so the interleave is static.
  - Copy/Identity ops kept off ACT so the sigmoid/tanh tables stay
    resident (table reloads cost 1.3us each at every chunk boundary).
  - States h, c stay fp32; fp16 copies (hTb, cw6) feed the PE.
  - Weights/constants pre-arranged host-side; event/vc/vn shipped fp16 in
    [feat, S, B] layout so each chunk loads with one contiguous DMA each.

Per-core layout is fully transposed (feature dims on SBUF partitions, batch
on the free dim):
  gates PSUM: 8 banks of [128, 8 blocks, 8 steps, 8 batch]; block order
  [g0 g1 f0 f1 i0 i1 o0 o1] (half = hidden half of HS=256).
"""

import numpy as np

B_FULL = 64
B_LOC = 8
N_CORES = 8
S = 4096
E, C, NN = 64, 32, 16
EMB, HS, EF, DIM = 128, 256, 128, 64
G4 = 4 * HS
MC = 64              # steps per micro-chunk (gates PSUM capacity)
P = 128

# block order (g,f,i,o) x (half0, half1) -> column offset into the
# [i f g o] gate layout of Wx/Wh/bias
BLK_COL = [2 * HS, 2 * HS + 128, HS, HS + 128, 0, 128, 3 * HS, 3 * HS + 128]


def build_nc(s_total=S, mc=MC):
    import concourse.bass as bass
    import concourse.tile as tile
    import concourse.mybir as mybir
    from concourse import bacc
    from concourse.bass import ds

    fp32 = mybir.dt.float32
    fp16 = mybir.dt.float16
    AF = mybir.ActivationFunctionType
    OP = mybir.AluOpType

    n_chunks = s_total // mc
    assert n_chunks % 2 == 0
    NCH_COLS = mc * B_LOC          # 512 cols per chunk (t-major, b-minor)

    nc = bacc.Bacc()

    # inputs already transposed/converted host-side
    event_d = nc.declare_dram_parameter("event", [E, s_total, B_LOC], fp16, isOutput=False)
    vc_d = nc.declare_dram_parameter("vc", [C, s_total, B_LOC], fp16, isOutput=False)
    vn_d = nc.declare_dram_parameter("vn", [NN, s_total, B_LOC], fp16, isOutput=False)
    h0_d = nc.declare_dram_parameter("h0", [P, 2, B_LOC], fp32, isOutput=False)
    h0h_d = nc.declare_dram_parameter("h0h", [P, 2, B_LOC], fp16, isOutput=False)
    c0_d = nc.declare_dram_parameter("c0", [P, 2, B_LOC], fp32, isOutput=False)
    cw0_d = nc.declare_dram_parameter("cw0", [P, 3, 2, B_LOC], fp16, isOutput=False)
    Wx_d = nc.declare_dram_parameter("Wx", [P, G4], fp16, isOutput=False)
    Wh_d = nc.declare_dram_parameter("Wh", [P, 2, G4], fp16, isOutput=False)
    ident_d = nc.declare_dram_parameter("ident", [P, P], fp16, isOutput=False)
    wcbc_d = nc.declare_dram_parameter("wcbc", [P, 3, 2, B_LOC], fp16, isOutput=False)
    bias8_d = nc.declare_dram_parameter("bias8", [8, P], fp16, isOutput=False)
    ind8_d = nc.declare_dram_parameter("ind8", [8, 8, mc // 8, B_LOC], fp16, isOutput=False)
    Ve_d = nc.declare_dram_parameter("Ve", [E, EMB], fp16, isOutput=False)
    Vc2_d = nc.declare_dram_parameter("Vc2", [C, EMB], fp16, isOutput=False)
    Vn_d = nc.declare_dram_parameter("Vn", [NN, EMB], fp16, isOutput=False)
    Wlin_d = nc.declare_dram_parameter("Wlin", [P, 2, DIM], fp32, isOutput=False)
    blin_d = nc.declare_dram_parameter("blin", [DIM, 1], fp32, isOutput=False)
    Wef1_d = nc.declare_dram_parameter("Wef1", [P, EF], fp16, isOutput=False)
    bef1_d = nc.declare_dram_parameter("bef1", [P, 1], fp32, isOutput=False)
    Wef3_d = nc.declare_dram_parameter("Wef3", [P, HS], fp16, isOutput=False)
    bef3_d = nc.declare_dram_parameter("bef3", [P, 2], fp32, isOutput=False)
    out_d = nc.declare_dram_parameter("out", [B_LOC, DIM], fp32, isOutput=True)

    with tile.TileContext(nc) as tc:
        with (
            tc.tile_pool(name="wts", bufs=1) as wts,
            tc.tile_pool(name="state", bufs=1) as stp,
            tc.tile_pool(name="chunk", bufs=1) as chp,
            tc.tile_pool(name="scr", bufs=3) as scr,
            tc.tile_pool(name="psum", bufs=1, space="PSUM") as psp,
        ):
            # ---------------- weights straight into SBUF ----------------
            def ldw(nm, shape, dt, src):
                t = wts.tile(shape, dt, name=nm, tag=nm)
                nc.sync.dma_start(t[:], src[:])
                return t

            Wh_sb = ldw("Wh_sb", [P, 2, G4], fp16, Wh_d)
            Wx_sb = ldw("Wx_sb", [P, G4], fp16, Wx_d)
            ident_sb = ldw("ident_sb", [P, P], fp16, ident_d)
            wcbc_sb = ldw("wcbc_sb", [P, 3, 2, B_LOC], fp16, wcbc_d)
            Ve_sb = ldw("Ve_sb", [E, EMB], fp16, Ve_d)
            Vc2_sb = ldw("Vc2_sb", [C, EMB], fp16, Vc2_d)
            Vn_sb = ldw("Vn_sb", [NN, EMB], fp16, Vn_d)
            Wef1_sb = ldw("Wef1_sb", [P, EF], fp16, Wef1_d)
            Wef3_sb = ldw("Wef3_sb", [P, HS], fp16, Wef3_d)
            bias8_sb = ldw("bias8_sb", [8, P], fp16, bias8_d)
            ind8_sb = ldw("ind8_sb", [8, 8, mc // 8, B_LOC], fp16, ind8_d)
            Wlin_sb = ldw("Wlin_sb", [P, 2, DIM], fp32, Wlin_d)
            blin_col = ldw("blin_col", [DIM, 1], fp32, blin_d)
            bef1_col = ldw("bef1_col", [P, 1], fp32, bef1_d)
            bef3_col = ldw("bef3_col", [P, 2], fp32, bef3_d)

            # ---------------- state (all owned by DVE in the scan) -------
            hT = stp.tile([P, 2, B_LOC], fp32)       # [p, half, b]
            hTb = stp.tile([P, 2, B_LOC], fp16)
            cw6 = stp.tile([P, 3, 2, B_LOC], fp16)
            # STATE = [c_hat(2,8) | c(2,8) | G(2,8)]
            STATE = stp.tile([P, 3, 2, B_LOC], fp32)
            nc.sync.dma_start(hT[:], h0_d[:])
            nc.sync.dma_start(hTb[:], h0h_d[:])
            nc.sync.dma_start(cw6[:], cw0_d[:])
            nc.sync.dma_start(STATE[:, 1, :, :], c0_d[:])

            # ---------------- ping-pong chunk tile sets ------------------
            def mkset(p):
                return dict(
                    evT=chp.tile([E, mc, B_LOC], fp16, name=f"evT{p}", tag=f"evT{p}"),
                    vcT=chp.tile([C, mc, B_LOC], fp16, name=f"vcT{p}", tag=f"vcT{p}"),
                    vnT=chp.tile([NN, mc, B_LOC], fp16, name=f"vnT{p}", tag=f"vnT{p}"),
                    s_sb=chp.tile([P, NCH_COLS], fp16, name=f"s_sb{p}", tag=f"s_sb{p}"),
                    tn_sb=chp.tile([P, NCH_COLS], fp32, name=f"tn_sb{p}", tag=f"tn_sb{p}"),
                    u_sb=chp.tile([P, NCH_COLS], fp16, name=f"u_sb{p}", tag=f"u_sb{p}"),
                    xTb=chp.tile([P, mc, B_LOC], fp16, name=f"xTb{p}", tag=f"xTb{p}"),
                    jmj=chp.tile([P, mc, 4, B_LOC], fp32, name=f"jmj{p}", tag=f"jmj{p}"),
                )

            SETS = [mkset(0), mkset(1)]
            banks = []
            for k in range(8):
                banks.append(psp.tile([P, 8, mc // 8, B_LOC], fp32,
                                      tag=f"bank{k}", name=f"bank{k}"))

            # ---------------- phase emitters (for chunk at t0e, set st) ---
            def ph_dma(st, t0e):
                nc.sync.dma_start(st["evT"][:], event_d[:, t0e, :])
                nc.sync.dma_start(st["vcT"][:], vc_d[:, t0e, :])
                nc.sync.dma_start(st["vnT"][:], vn_d[:, t0e, :])

            def ph_a1(st):
                ps_x = banks[0][:].rearrange("p blk t b -> p (blk t b)")
                ps_h = banks[1][:].rearrange("p blk t b -> p (blk t b)")
                nc.tensor.matmul(ps_x, Ve_sb[:],
                                 st["evT"][:].rearrange("e t b -> e (t b)"),
                                 start=True, stop=True, skip_group_check=True)
                nc.vector.tensor_copy(st["s_sb"][:], ps_x)
                nc.tensor.matmul(ps_x, Vc2_sb[:],
                                 st["vcT"][:].rearrange("c t b -> c (t b)"),
                                 start=False, stop=True, skip_group_check=True)
                nc.tensor.matmul(ps_h, Vn_sb[:],
                                 st["vnT"][:].rearrange("n t b -> n (t b)"),
                                 start=True, stop=True, skip_group_check=True)
                nc.scalar.activation(st["tn_sb"][:], ps_h, AF.Tanh)
                nc.vector.scalar_tensor_tensor(
                    st["xTb"][:].rearrange("p t b -> p (t b)"),
                    st["tn_sb"][:], 2.0, ps_x, op0=OP.mult, op1=OP.add)

            def ph_a2(st):
                ps_h = banks[1][:].rearrange("p blk t b -> p (blk t b)")
                nc.tensor.matmul(ps_h, Wef1_sb[:], st["s_sb"][:],
                                 start=True, stop=True, skip_group_check=True)
                nc.scalar.activation(st["u_sb"][:], ps_h, AF.Tanh,
                                     bias=bef1_col[:, 0:1])

            def ph_a3(st, hf):
                ps_j = banks[2 + hf][:].rearrange("p blk t b -> p (blk t b)")
                nc.tensor.matmul(ps_j, Wef3_sb[:, hf * P:(hf + 1) * P], st["u_sb"][:],
                                 start=True, stop=True, skip_group_check=True)
                nc.scalar.activation(st["jmj"][:, :, hf, :], ps_j, AF.Sigmoid,
                                     bias=bef3_col[:, hf:hf + 1])

            def ph_mj(st):
                nc.vector.tensor_scalar(st["jmj"][:, :, 2:4, :],
                                        st["jmj"][:, :, 0:2, :],
                                        -1.0, 1.0, op0=OP.mult, op1=OP.add)

            def ph_b(st, k):
                nc.tensor.matmul(
                    banks[k][:].rearrange("p blk t b -> p (blk t b)"),
                    bias8_sb[:], ind8_sb[:].rearrange("j blk t b -> j (blk t b)"),
                    start=True, stop=False, skip_group_check=True)
                for blk in range(8):
                    co = BLK_COL[blk]
                    nc.tensor.matmul(
                        banks[k][:, blk, :, :], Wx_sb[:, co:co + P],
                        st["xTb"][:, (mc // 8) * k:(mc // 8) * k + mc // 8, :],
                        start=False, stop=False, skip_group_check=True)

            def phase_all(st, t0e):
                ph_dma(st, t0e)
                ph_a1(st)
                ph_a2(st)
                ph_a3(st, 0)
                ph_a3(st, 1)
                ph_mj(st)
                for k in range(8):
                    ph_b(st, k)

            # ---------------- one scan step --------------------------------
            def scan_step(st, tl):
                bk = banks[tl // 8]
                trow = tl % 8
                jmj_t = st["jmj"][:, tl, :, :]

                m2T = scr.tile([P, 2, B_LOC], fp32, tag="m2T")
                nc.vector.tensor_mul(m2T[:], jmj_t[:, 2:4, :], hT[:])

                for blk in range(8):
                    co = BLK_COL[blk]
                    for k in range(2):
                        nc.tensor.matmul(
                            bk[:, blk, trow, :], Wh_sb[:, k, co:co + P],
                            hTb[:, k, :],
                            start=False, stop=(blk < 2 and k == 1),
                            skip_group_check=True)
                nc.tensor.matmul(
                    bk[:, 2:8, trow, :], ident_sb[:],
                    cw6[:].rearrange("p r hf b -> p (r hf b)"),
                    start=False, stop=True, skip_group_check=True)

                S8 = scr.tile([P, 8, B_LOC], fp32, tag="S8")
                nc.scalar.activation(S8[:], bk[:, :, trow, :], AF.Sigmoid)

                nc.vector.tensor_scalar(STATE[:, 2, :, :], S8[:, 0:2, :],
                                        2.0, -1.0, op0=OP.mult, op1=OP.add)
                fcig = scr.tile([P, 4, B_LOC], fp32, tag="fcig")
                nc.vector.tensor_mul(
                    fcig[:], S8[:, 2:6, :],
                    STATE[:, 1:3, :, :].rearrange("p s hf b -> p (s hf) b"))
                nc.vector.tensor_add(STATE[:, 0, :, :], fcig[:, 0:2, :], fcig[:, 2:4, :])
                joT = scr.tile([P, 2, B_LOC], fp32, tag="joT")
                nc.vector.tensor_mul(joT[:], jmj_t[:, 0:2, :], S8[:, 6:8, :])
                jcmj = scr.tile([P, 4, B_LOC], fp32, tag="jcmj")
                nc.vector.tensor_mul(
                    jcmj[:], jmj_t[:],
                    STATE[:, 0:2, :, :].rearrange("p s hf b -> p (s hf) b"))
                thT = scr.tile([P, 2, B_LOC], fp32, tag="thT")
                nc.scalar.activation(thT[:], STATE[:, 0, :, :], AF.Tanh)
                m1T = scr.tile([P, 2, B_LOC], fp32, tag="m1T")
                nc.vector.tensor_mul(m1T[:], joT[:], thT[:])
                nc.vector.tensor_add(hTb[:], m1T[:], m2T[:])
                # post-critical tail, still DVE
                nc.vector.tensor_add(STATE[:, 1, :, :], jcmj[:, 0:2, :], jcmj[:, 2:4, :])
                nc.vector.tensor_mul(
                    cw6[:],
                    STATE[:, 1, :, :].unsqueeze(1).to_broadcast([P, 3, 2, B_LOC]),
                    wcbc_sb[:])
                nc.vector.tensor_add(hT[:], m1T[:], m2T[:])

            # scan chunk reading set `sp`, interleaving phase for the next
            # chunk into set 1-sp (next_t0e None => no interleave)
            def scan_chunk(sp, next_t0e):
                st = SETS[sp]
                nst = SETS[1 - sp]
                if next_t0e is not None:
                    ph_dma(nst, next_t0e)
                for tl in range(mc):
                    scan_step(st, tl)
                    if next_t0e is None:
                        continue
                    if tl == 15:
                        ph_a1(nst)
                    elif tl == 23:
                        ph_a2(nst)
                    elif tl == 31:
                        ph_a3(nst, 0)
                    elif tl == 35:
                        ph_a3(nst, 1)
                    elif tl == 39:
                        ph_mj(nst)
                        ph_b(nst, 0)
                        ph_b(nst, 1)
                    elif tl == 47:
                        ph_b(nst, 2)
                        ph_b(nst, 3)
                        ph_b(nst, 4)
                    elif tl == 55:
                        ph_b(nst, 5)
                        ph_b(nst, 6)
                    elif tl == 63:
                        ph_b(nst, 7)

            # ---------------- schedule -------------------------------------
            # prologue: phase(0)
            phase_all(SETS[0], ds(0, mc))
            # loop covers scans 0..n-3 (2 per iteration), phases 1..n-2
            n_iter = (n_chunks - 2) // 2
            with tc.For_i(0, n_iter, 1, staggered_reset=True,
                          hint_engines=(mybir.EngineType.PE,
                                        mybir.EngineType.Activation,
                                        mybir.EngineType.DVE,
                                        mybir.EngineType.Pool)) as m:
                scan_chunk(0, ds(m * (2 * mc) + mc, mc))
                scan_chunk(1, ds(m * (2 * mc) + 2 * mc, mc))
            # epilogue: scan(n-2) + phase(n-1), scan(n-1)
            scan_chunk(0, ds((n_chunks - 1) * mc, mc))
            scan_chunk(1, None)

            # ---------------- output projection ----------------
            ps_o = psp.tile([DIM, B_LOC], fp32, tag="bank0")
            for k in range(2):
                nc.tensor.matmul(ps_o[:], Wlin_sb[:, k, :], hT[:, k, :],
                                 start=(k == 0), stop=(k == 1))
            outT = stp.tile([DIM, B_LOC], fp32)
            nc.scalar.activation(outT[:], ps_o[:], AF.Identity, bias=blin_col[:, 0:1])
            nc.sync.dma_start(out_d.rearrange("b d -> d b"), outT[:])

    nc.finalize()
    return nc


_NC_CACHE = {}


def _get_nc(s_total=S, mc=MC):
    key = (s_total, mc)
    if key not in _NC_CACHE:
        _NC_CACHE[key] = build_nc(s_total, mc)
    return _NC_CACHE[key]


def _prep_shared(inputs):
    """Host-side constant prep shared by all cores."""
    f16 = np.float16
    f32 = np.float32
    Wh = np.asarray(inputs["Wh"], f32).copy()    # [256, 1024]
    Wx = np.asarray(inputs["Wx"], f32).copy()
    Wc = np.asarray(inputs["Wc"], f32)
    bias = np.asarray(inputs["bias"], f32).copy()
    # g-block pre-activations scaled x2: tanh(x) = 2*sigmoid(2x) - 1
    Wh[:, 2 * HS:3 * HS] *= 2.0
    Wx[:, 2 * HS:3 * HS] *= 2.0
    bias[2 * HS:3 * HS] *= 2.0
    sh = {
        "Wh": np.ascontiguousarray(
            Wh.reshape(2, P, G4).transpose(1, 0, 2)).astype(f16),
        "Wx": np.ascontiguousarray(Wx).astype(f16),
        "Ve": np.ascontiguousarray(inputs["Ve"]).astype(f16),
        "Vc2": np.ascontiguousarray(2.0 * np.asarray(inputs["Vc"], f32)).astype(f16),
        "Vn": np.ascontiguousarray(inputs["Vn"]).astype(f16),
        "Wef1": np.ascontiguousarray(inputs["Wef1"]).astype(f16),
        "Wef3": np.ascontiguousarray(inputs["Wef3"]).astype(f16),
        "Wlin": np.ascontiguousarray(
            np.asarray(inputs["Wlin"], f32).reshape(2, P, DIM).transpose(1, 0, 2)),
        "blin": np.ascontiguousarray(np.asarray(inputs["blin"], f32).reshape(DIM, 1)),
        "bef1": np.ascontiguousarray(np.asarray(inputs["bef1"], f32).reshape(P, 1)),
        "bef3": np.ascontiguousarray(
            np.asarray(inputs["bef3"], f32).reshape(2, P).T),
    }
    # bias8[blk, p] = bias[BLK_COL[blk] + p] (block order)
    bias8 = np.stack([bias[co:co + P] for co in BLK_COL])
    sh["bias8"] = np.ascontiguousarray(bias8).astype(f16)
    # ind8[r, blk, t, b] = (r == blk)
    ind8 = np.zeros((8, 8, MC // 8, B_LOC), f32)
    for r in range(8):
        ind8[r, r] = 1.0
    sh["ind8"] = ind8.astype(f16)
    sh["ident"] = np.eye(P, dtype=f16)
    # wcbc[p, r, hf, b] = Wc[row_r, hf*128+p] for rows [f,i,o]
    wcbc = np.zeros((P, 3, 2, B_LOC), f32)
    for r, wr in enumerate([1, 0, 2]):
        for hf in range(2):
            wcbc[:, r, hf, :] = Wc[wr, hf * P:(hf + 1) * P][:, None]
    sh["wcbc"] = wcbc.astype(f16)
    sh["_wcbc_f32"] = wcbc
    return sh


def _make_in_maps(inputs, s_total=S):
    f16 = np.float16
    f32 = np.float32
    sh = _prep_shared(inputs)
    ev = np.asarray(inputs["event"], f32)[:, :s_total]   # [B, S, E]
    vc = np.asarray(inputs["vc"], f32)[:, :s_total]
    vn = np.asarray(inputs["vn"], f32)[:, :s_total]
    h0 = np.asarray(inputs["h0"], f32)                   # [B, 256]
    c0 = np.asarray(inputs["c0"], f32)
    per_core = []
    for i in range(N_CORES):
        sl = slice(i * B_LOC, (i + 1) * B_LOC)
        h0T = np.ascontiguousarray(
            h0[sl].reshape(B_LOC, 2, P).transpose(2, 1, 0))   # [P, 2, B]
        c0T = np.ascontiguousarray(
            c0[sl].reshape(B_LOC, 2, P).transpose(2, 1, 0))
        m = dict(sh)
        m["event"] = np.ascontiguousarray(
            ev[sl].transpose(2, 1, 0)).astype(f16)            # [E, S, B]
        m["vc"] = np.ascontiguousarray(vc[sl].transpose(2, 1, 0)).astype(f16)
        m["vn"] = np.ascontiguousarray(vn[sl].transpose(2, 1, 0)).astype(f16)
        m["h0"] = h0T
        m["h0h"] = h0T.astype(f16)
        m["c0"] = c0T
        m["cw0"] = (c0T[:, None, :, :] * sh["_wcbc_f32"]).astype(f16)
        per_core.append(m)
    return per_core


def run(inputs, s_total=S, mc=MC, trace=False):
    """Returns (out [B_FULL, DIM], exec_time_ns or None)."""
    from concourse.bass_utils import run_bass_kernel_spmd

    nc = _get_nc(s_total, mc)
    in_maps = _make_in_maps(inputs, s_total)
    res = run_bass_kernel_spmd(nc, in_maps, list(range(N_CORES)), trace=trace)
    out = np.concatenate([res.results[i]["out"] for i in range(N_CORES)], axis=0)
    return out, res.exec_time_ns


def kernel(**inputs):
    out, _ = run(inputs)
    return out
